# revision 1
# baseline (speedup 1.0000x reference)
"""GQA attention kernel for 8 trn2 NeuronCores (tensor-parallel over heads).

Problem: B=1, S=2048, D=2048, NQ=32 q heads, NKV=8 kv heads, HD=64.
Core i handles q heads 4i..4i+3 and kv head i; out = sum of per-core partials.

Layout strategy (all transposed, zero P-matrix transposes):
  x^T tiles  built on device via PE transpose (fp32 has no DMA transpose)
  Q^T [128=2 heads x 64, S] per head-pair, K^T [64, S]   (proj matmuls)
  V   [S, 64+1] normal layout + ones column (softmax sums come free from PV)
  S^T block = K^T_slice.T @ Q^T  -> exp on ACT -> PV: V_ext.T @ expS^T
  row 64 of PV psum = softmax denominators; normalize via K=1 bcast matmul
  out-proj: lhsT = O^T directly (no transpose), partial written to DRAM
RMSNorm over head dim (= partitions) via ones-selector matmuls on PE.
"""

import os
import sys

sys.path.insert(0, "/opt/trn_rl_repo")

import numpy as np

S = 2048
D = 2048
HD = 64
NQ = 32
NKV = 8
P = 128
EPS = 1e-6
SCALE = 0.125  # 1/sqrt(HD)
N_CORES = 8

_CACHE = {}
LAST_RESULTS = None


def _build_nc():
    import concourse.bass as bass
    import concourse.tile as tile
    from concourse import bacc, mybir

    f32 = mybir.dt.float32
    nc = bacc.Bacc("TRN2", target_bir_lowering=False, debug=False)

    def dram_in(name, shape):
        return nc.dram_tensor(name, list(shape), f32, kind="ExternalInput").ap()

    io = {
        "x2d": dram_in("x2d", (S, D)),
        "wqa": dram_in("wqa", (P, 16, P)),
        "wqb": dram_in("wqb", (P, 16, P)),
        "wk": dram_in("wk", (P, 16, HD)),
        "wv": dram_in("wv", (P, 16, HD)),
        "wo": dram_in("wo", (P, 2, D)),
        "cos4": dram_in("cos4", (P, S)),
        "sin4s": dram_in("sin4s", (P, S)),
        "gq2": dram_in("gq2", (P, 1)),
        "gk": dram_in("gk", (HD, 1)),
        "maskz": dram_in("maskz", (P, 1024)),
        "ones2": dram_in("ones2", (P, 2)),
        "ones64": dram_in("ones64", (1, HD)),
        "rot2": dram_in("rot2", (P, P)),
        "ones65": dram_in("ones65", (HD + 1, HD)),
        "ident": dram_in("ident", (P, P)),
        "out": nc.dram_tensor("out", [S, D], f32, kind="ExternalOutput").ap(),
    }

    from contextlib import ExitStack

    with tile.TileContext(nc) as tc, ExitStack() as ctx:
        _emit(ctx, tc, io, bass, mybir)
    nc.compile()
    return nc


def _emit(ctx, tc, io, bass, mybir):
    nc = tc.nc
    f32 = mybir.dt.float32
    Exp = mybir.ActivationFunctionType.Exp
    Sqrt = mybir.ActivationFunctionType.Sqrt
    mult = mybir.AluOpType.mult

    cpool = ctx.enter_context(tc.tile_pool(name="consts", bufs=1))
    pers = ctx.enter_context(tc.tile_pool(name="persist", bufs=1))

    # ---- constants / weights into SBUF ----
    def cload(name, shape):
        t = cpool.tile(list(shape), f32, tag=name, name=name)
        nc.sync.dma_start(t[:], io[name][:])
        return t

    wqa = cload("wqa", (P, 16, P))
    wqb = cload("wqb", (P, 16, P))
    wk = cload("wk", (P, 16, HD))
    wv = cload("wv", (P, 16, HD))
    wo = cload("wo", (P, 2, D))
    cos4 = cload("cos4", (P, S))
    sin4s = cload("sin4s", (P, S))
    gq2 = cload("gq2", (P, 1))
    gk = cload("gk", (HD, 1))
    maskz = cload("maskz", (P, 1024))
    ones2 = cload("ones2", (P, 2))
    ones64 = cload("ones64", (1, HD))
    rot2 = cload("rot2", (P, P))
    ones65 = cload("ones65", (HD + 1, HD))
    ident = cload("ident", (P, P))

    # ---- persistent activations ----
    QT = [pers.tile([P, S], f32, tag=f"qt{t}", name=f"QT{t}") for t in range(2)]  # head pairs
    KT = pers.tile([P, S], f32, tag="kt")  # rows 64-127 = duplicate of 0-63
    V = pers.tile([P, 16, HD + 1], f32, tag="v")  # [seq128, kblock, hd+ones]
    OT = pers.tile([P, 2, S], f32, tag="ot")  # attn out transposed
    stdq = [pers.tile([2, S], f32, tag=f"stdq{t}", name=f"stdq{t}") for t in range(2)]
    stdk = pers.tile([1, S], f32, tag="stdk")

    nc.vector.memset(V[:, :, HD : HD + 1], 1.0)
    epsc = pers.tile([P, 1], f32, tag="epsc")
    nc.vector.memset(epsc[:], EPS)

    # ================= Phase 1: transpose x + projections =================
    with (
        tc.tile_pool(name="xraw", bufs=2) as xrp,
        tc.tile_pool(name="xt", bufs=1) as xtp,
        tc.tile_pool(name="sq", bufs=2) as sqp,
        tc.tile_pool(name="tpsum", bufs=3, space="PSUM") as tp,
        tc.tile_pool(name="ppsum", bufs=2, space="PSUM") as pp,
        tc.tile_pool(name="vpsum", bufs=2, space="PSUM") as vp,
        tc.tile_pool(name="sspsum", bufs=1, space="PSUM") as ssp,
    ):
        for sc in range(4):  # seq chunks of 512
            xT = xtp.tile([P, 16, 512], f32, tag="xt")
            for sb in range(4):  # 128-row blocks
                xraw = xrp.tile([P, D], f32, tag="xraw")
                r0 = sc * 512 + sb * P
                nc.sync.dma_start(xraw[:], io["x2d"][r0 : r0 + P, :])
                for kc in range(16):
                    pt = tp.tile([P, P], f32, tag="t")
                    nc.tensor.transpose(pt[:], xraw[:, kc * P : (kc + 1) * P], ident[:])
                    nc.any.tensor_copy(xT[:, kc, sb * P : (sb + 1) * P], pt[:])

            cs = slice(sc * 512, (sc + 1) * 512)

            def proj(lhsT_w, m, dst_copy, ss_dst=None, n_ones=None):
                ps = pp.tile([P, 512], f32, tag="p", name="ps")[:m]
                for kc in range(16):
                    nc.tensor.matmul(
                        ps, lhsT_w[:, kc, :], xT[:, kc, :],
                        start=(kc == 0), stop=(kc == 15),
                    )
                dst_copy(ps)
                if ss_dst is not None:
                    sq = sqp.tile([P, 512], f32, tag="sq", name="sq")[:m]
                    nc.scalar.activation(sq, ps, mybir.ActivationFunctionType.Square)
                    nm = ss_dst.shape[0]
                    ssps = ssp.tile([2, 512], f32, tag="ss", name="ssps")[:nm]
                    nc.tensor.matmul(ssps, n_ones, sq, start=True, stop=True)
                    # std = sqrt(mean + eps)
                    nc.scalar.activation(ss_dst, ssps, Sqrt, bias=epsc[:nm], scale=1.0 / HD)

            proj(wqa, P, lambda ps: nc.vector.tensor_copy(QT[0][:, cs], ps),
                 ss_dst=stdq[0][:, cs], n_ones=ones2[:, :])
            proj(wqb, P, lambda ps: nc.vector.tensor_copy(QT[1][:, cs], ps),
                 ss_dst=stdq[1][:, cs], n_ones=ones2[:, :])
            proj(wk, HD, lambda ps: nc.vector.tensor_copy(KT[0:HD, cs], ps),
                 ss_dst=stdk[:, cs], n_ones=ones2[:HD, 0:1])
            # V in normal layout: lhsT = x^T slice, rhs = wv
            for ms in range(4):
                pv = vp.tile([P, HD], f32, tag="v")
                for kc in range(16):
                    nc.tensor.matmul(
                        pv[:], xT[:, kc, ms * P : (ms + 1) * P], wv[:, kc, :],
                        start=(kc == 0), stop=(kc == 15),
                    )
                nc.any.tensor_copy(V[:, sc * 4 + ms, 0:HD], pv[:])

    # ================= Phase 2: RMSNorm + RoPE (in place) =================
    with (
        tc.tile_pool(name="rtmp", bufs=2) as rtp,
        tc.tile_pool(name="rsm", bufs=2) as rsp,
        tc.tile_pool(name="bcpsum", bufs=2, space="PSUM") as bcp,
        tc.tile_pool(name="swpsum", bufs=2, space="PSUM") as swp,
        tc.tile_pool(name="selp", bufs=1, space="PSUM") as selpp,
    ):
        # selector for head-pair bcast: sel2 [2, P] = ones2.T (via PE transpose)
        selps = selpp.tile([2, P], f32, tag="sel")
        nc.tensor.transpose(selps[:], ones2[:, :], ident[:])
        sel2 = rsp.tile([2, P], f32, tag="sel2")
        nc.vector.tensor_copy(sel2[:], selps[:])

        def norm_rope(T, std, g, sel, m):
            # T [m, S], heads on 64-row groups; std [nh, S]; all base partition 0
            nh = std.shape[0]
            rstd = rsp.tile([2, S], f32, tag="rstd", name="rstd")[:nh]
            nc.vector.reciprocal(rstd, std)
            tmpc = rtp.tile([P, S], f32, tag="tc", name="tmpc")[:m]
            for c in range(4):
                cs = slice(c * 512, (c + 1) * 512)
                bc = bcp.tile([P, 512], f32, tag="bc", name="bc")[:m]
                nc.tensor.matmul(bc, sel, rstd[:, cs], start=True, stop=True)
                # T = (T * g) * bcast(rstd)   in place
                nc.vector.scalar_tensor_tensor(T[:, cs], T[:, cs], g, bc, mult, mult)
            nc.vector.tensor_mul(tmpc, T, cos4[:m, :])
            for c in range(4):
                cs = slice(c * 512, (c + 1) * 512)
                sw = swp.tile([P, 512], f32, tag="sw", name="sw")[:m]
                nc.tensor.matmul(sw, rot2[:m, :m], T[:, cs], start=True, stop=True)
                # T_chunk = swap(T_normed) * sin4s  (tmpc added after loop)
                nc.vector.tensor_mul(T[:, cs], sw, sin4s[:m, cs])
            nc.vector.tensor_add(T, T, tmpc)

        norm_rope(QT[0][:, :], stdq[0][:, :], gq2[:, :], sel2[:, :], P)
        norm_rope(QT[1][:, :], stdq[1][:, :], gq2[:, :], sel2[:, :], P)
        norm_rope(KT[0:HD, :], stdk[:, :], gk[:, :], ones64[:, :], HD)
        # duplicate normed+roped K into partitions 64-127 (for odd-head scores)
        nc.sync.dma_start(KT[HD:P, :], KT[0:HD, :])

    # ================= Phase 3: attention + out-projection =================
    with (
        tc.tile_pool(name="exps", bufs=3) as ep,
        tc.tile_pool(name="bcs", bufs=2) as bcsp,
        tc.tile_pool(name="ov", bufs=2) as ovp,
        tc.tile_pool(name="spsum", bufs=3, space="PSUM") as sp,
        tc.tile_pool(name="opsum", bufs=2, space="PSUM") as op_,
        tc.tile_pool(name="bpsum", bufs=1, space="PSUM") as bp,
        tc.tile_pool(name="oppsum", bufs=2, space="PSUM") as opp,
    ):
        for qc in range(4):
            qs = slice(qc * 512, (qc + 1) * 512)
            for h in range(4):
                pair, poff = h // 2, (h % 2) * HD
                Q = QT[pair]
                nkb = 4 * qc + 4
                po = op_.tile([HD + 1, 512], f32, tag="o")

                def score_exp(kb):
                    ps = sp.tile([P, 512], f32, tag="s")
                    nc.tensor.matmul(
                        ps,
                        KT[poff : poff + HD, kb * P : (kb + 1) * P],
                        Q[poff : poff + HD, qs],
                        start=True, stop=True,
                    )
                    es = ep.tile([P, 512], f32, tag="e")
                    nc.scalar.activation(es, ps, Exp, scale=SCALE)
                    o = kb - 4 * qc
                    if o >= 0:
                        mz = maskz[:, 512 - o * P : 1024 - o * P]
                        nc.vector.tensor_mul(es, es, mz)
                    return es

                def pv(kb, es):
                    nc.tensor.matmul(
                        po, V[:, kb, :], es,
                        start=(kb == 0), stop=(kb == nkb - 1),
                    )

                prev = score_exp(0)
                for kb in range(1, nkb):
                    cur = score_exp(kb)
                    pv(kb - 1, prev)
                    prev = cur
                pv(nkb - 1, prev)

                # normalize: row HD of po holds the softmax denominators
                rec = bcsp.tile([HD + 1, 512], f32, tag="rec", name="rec")[HD : HD + 1]
                nc.vector.reciprocal(rec, po[HD : HD + 1, :])
                bc = bp.tile([HD, 512], f32, tag="b")
                nc.tensor.matmul(bc, ones65[HD : HD + 1, :], rec, start=True, stop=True)
                bcs = bcsp.tile([HD, 512], f32, tag="bcs")
                nc.vector.tensor_copy(bcs, bc)
                if poff == 0:
                    nc.vector.tensor_mul(OT[0:HD, pair, qs], po[0:HD, :], bcs)
                else:
                    stg = bcsp.tile([HD, 512], f32, tag="stg")
                    nc.vector.tensor_mul(stg, po[0:HD, :], bcs)
                    nc.sync.dma_start(OT[HD:P, pair, qs], stg[:])

            # out-projection for this q chunk (all 4 heads now done)
            for ms in range(4):
                sl = slice(qc * 512 + ms * P, qc * 512 + (ms + 1) * P)
                for dc in range(4):
                    pso = opp.tile([P, 512], f32, tag="op")
                    for kc in range(2):
                        nc.tensor.matmul(
                            pso, OT[:, kc, sl], wo[:, kc, dc * 512 : (dc + 1) * 512],
                            start=(kc == 0), stop=(kc == 1),
                        )
                    ov = ovp.tile([P, 512], f32, tag="ov")
                    nc.vector.tensor_copy(ov[:], pso[:])
                    nc.sync.dma_start(io["out"][sl, dc * 512 : (dc + 1) * 512], ov[:])


def _prep_core_inputs(i, x, cos, sin, g_q, g_k, Wq, Wk, Wv, Wo):
    c0 = i * 4 * HD
    k0 = i * HD
    wqa = np.ascontiguousarray(
        Wq[:, c0 : c0 + P].reshape(16, P, P).transpose(1, 0, 2))
    wqb = np.ascontiguousarray(
        Wq[:, c0 + P : c0 + 2 * P].reshape(16, P, P).transpose(1, 0, 2))
    wk = np.ascontiguousarray(
        Wk[:, k0 : k0 + HD].reshape(16, P, HD).transpose(1, 0, 2))
    wv = np.ascontiguousarray(
        Wv[:, k0 : k0 + HD].reshape(16, P, HD).transpose(1, 0, 2))
    wo = np.ascontiguousarray(
        Wo[c0 : c0 + 2 * P, :].reshape(2, P, D).transpose(1, 0, 2))
    cosT = cos.T.astype(np.float32)  # [32, S]
    sinT = sin.T.astype(np.float32)
    cos4 = np.tile(cosT, (4, 1))
    sin4s = np.concatenate([-sinT, sinT, -sinT, sinT], axis=0)
    gq2 = np.tile(g_q, 2)[:, None].astype(np.float32)
    gk = g_k[:, None].astype(np.float32)
    tri = np.triu(np.ones((P, P), dtype=np.float32))  # [k within blk, q within blk]
    mask0 = np.concatenate([tri, np.ones((P, 384), dtype=np.float32)], axis=1)
    maskz = np.concatenate([np.zeros((P, 512), dtype=np.float32), mask0], axis=1)
    ones2 = np.zeros((P, 2), dtype=np.float32)
    ones2[:HD, 0] = 1.0
    ones2[HD:, 1] = 1.0
    r64 = np.roll(np.eye(HD, dtype=np.float32), 32, axis=0)
    rot2 = np.zeros((P, P), dtype=np.float32)
    rot2[:HD, :HD] = r64
    rot2[HD:, HD:] = r64
    return {
        "x2d": np.ascontiguousarray(x.reshape(S, D)),
        "wqa": wqa, "wqb": wqb, "wk": wk, "wv": wv, "wo": wo,
        "cos4": np.ascontiguousarray(cos4), "sin4s": np.ascontiguousarray(sin4s),
        "gq2": gq2, "gk": gk, "maskz": maskz, "ones2": ones2,
        "ones64": np.ones((1, HD), dtype=np.float32),
        "rot2": rot2,
        "ones65": np.ones((HD + 1, HD), dtype=np.float32),
        "ident": np.eye(P, dtype=np.float32),
    }


def kernel(x, cos, sin, g_q, g_k, Wq, Wk, Wv, Wo):
    global LAST_RESULTS
    from concourse.bass_utils import run_bass_kernel_spmd

    if "nc" not in _CACHE:
        _CACHE["nc"] = _build_nc()
    nc = _CACHE["nc"]

    args = [np.asarray(a, dtype=np.float32) for a in
            (x, cos, sin, g_q, g_k, Wq, Wk, Wv, Wo)]
    in_maps = [_prep_core_inputs(i, *args) for i in range(N_CORES)]
    trace = bool(os.environ.get("BASS_TRACE"))
    res = run_bass_kernel_spmd(nc, in_maps, list(range(N_CORES)), trace=trace)
    LAST_RESULTS = res
    out = np.zeros((S, D), dtype=np.float32)
    for r in res.results:
        out += r["out"]
    return out.reshape(1, S, D)



# revision 9
# speedup vs baseline: 3.5947x; 3.5947x over previous
"""GQA attention kernel for 8 trn2 NeuronCores (tensor-parallel over heads).

Problem: B=1, S=2048, D=2048, NQ=32 q heads, NKV=8 kv heads, HD=64.
Core i handles q heads 4i..4i+3 and kv head i; out = sum of per-core partials.

v2: all-bf16 matmuls (1 cycle/row vs 4 for fp32 on the PE), x pre-transposed
on the host (kills 256 on-device PE transposes), proj+RMSNorm+RoPE fused per
512-column chunk, ACT stays on the exp table for the whole attention phase,
reciprocals via the fast custom-DVE op, psum->sbuf copies on the Pool engine,
out-projection matmuls interleaved into the attention stream to keep the PE
fed while ACT works through the exps.

Layout (all transposed, zero on-device transposes):
  xT   [128, 16, 2048] bf16  built on host: xT[p, kc, s] = x[s, 128*kc+p]
  Q^T  [128 = 2 heads x 64, S] per head-pair  (lhsT = Wq slice as stored)
  K^T  [64, S] normed+roped, duplicated into partitions 64..127
  V    [128 seq, 16 blocks, 64+1] with a ones column (softmax denominators
       fall out of the PV matmul as row 64)
  S^T block = K^T_slice.T @ Q^T -> exp on ACT -> PV: V_ext.T @ expS^T
  out-proj: lhsT = O^T directly, partial written to DRAM in bf16

RMSNorm over the head dim (= partitions) via ones-selector matmuls; the
per-head g vector is folded into the rstd-broadcast selector on the host.
"""

import os
import sys

sys.path.insert(0, "/opt/trn_rl_repo")

import numpy as np

try:
    import ml_dtypes

    BF = ml_dtypes.bfloat16
except ImportError:  # pragma: no cover
    BF = np.float32

S = 2048
D = 2048
HD = 64
NQ = 32
NKV = 8
P = 128
EPS = 1e-6
SCALE = 0.125  # 1/sqrt(HD)
N_CORES = 8

_CACHE = {}
LAST_RESULTS = None


def _build_nc():
    import concourse.bass as bass
    import concourse.tile as tile
    from concourse import bacc, mybir

    f32 = mybir.dt.float32
    bf16 = mybir.dt.bfloat16
    nc = bacc.Bacc("TRN2", target_bir_lowering=False, debug=False)

    def dram_in(name, shape, dt):
        return nc.dram_tensor(name, list(shape), dt, kind="ExternalInput").ap()

    io = {
        "xt": dram_in("xt", (P, 16, S), bf16),
        "wqa": dram_in("wqa", (P, 16, P), bf16),
        "wqb": dram_in("wqb", (P, 16, P), bf16),
        "wk": dram_in("wk", (P, 16, HD), bf16),
        "wv": dram_in("wv", (P, 16, HD), bf16),
        "wo": dram_in("wo", (P, 2, D), bf16),
        "cos4": dram_in("cos4", (P, S), bf16),
        "sin4s": dram_in("sin4s", (P, S), bf16),
        "tri": dram_in("tri", (P, P), bf16),
        "ones2": dram_in("ones2", (P, 2), bf16),
        "onesk": dram_in("onesk", (HD, 1), bf16),
        "sel2g": dram_in("sel2g", (2, P), bf16),
        "selk": dram_in("selk", (1, HD), bf16),
        "sel1": dram_in("sel1", (1, HD), bf16),
        "rot2": dram_in("rot2", (P, P), bf16),
        "out": nc.dram_tensor("out", [S, D], bf16, kind="ExternalOutput").ap(),
    }

    from contextlib import ExitStack

    with tile.TileContext(nc) as tc, ExitStack() as ctx:
        _emit(ctx, tc, io, bass, mybir)
    nc.compile()
    return nc


def _emit(ctx, tc, io, bass, mybir):
    nc = tc.nc
    f32 = mybir.dt.float32
    bf16 = mybir.dt.bfloat16
    Exp = mybir.ActivationFunctionType.Exp
    Sqrt = mybir.ActivationFunctionType.Sqrt
    Square = mybir.ActivationFunctionType.Square
    mult = mybir.AluOpType.mult

    cpool = ctx.enter_context(tc.tile_pool(name="consts", bufs=1))
    pers = ctx.enter_context(tc.tile_pool(name="persist", bufs=1))

    # ---- constants / weights into SBUF (DMA order = need order) ----
    def cload(name, shape, dt=bf16):
        t = cpool.tile(list(shape), dt, tag=name, name=name)
        nc.sync.dma_start(t[:], io[name][:])
        return t

    wqa = cload("wqa", (P, 16, P))
    wqb = cload("wqb", (P, 16, P))
    wk = cload("wk", (P, 16, HD))
    wv = cload("wv", (P, 16, HD))
    ones2 = cload("ones2", (P, 2))
    onesk = cload("onesk", (HD, 1))
    sel2g = cload("sel2g", (2, P))
    selk = cload("selk", (1, HD))
    sel1 = cload("sel1", (1, HD))
    rot2 = cload("rot2", (P, P))
    cos4 = cload("cos4", (P, S))
    sin4s = cload("sin4s", (P, S))
    tri = cload("tri", (P, P))
    wo = cload("wo", (P, 2, D))

    # ---- persistent activations ----
    QT = [pers.tile([P, S], bf16, tag=f"qt{t}", name=f"QT{t}") for t in range(2)]
    KT = pers.tile([P, S], bf16, tag="kt")  # rows 64-127 = copy of rows 0-63
    V = pers.tile([P, 16, HD + 1], bf16, tag="v")
    OT = pers.tile([P, 2, S], bf16, tag="ot")

    nc.vector.memset(V[:, :, HD : HD + 1], 1.0)
    epsc = pers.tile([P, 1], f32, tag="epsc")
    nc.vector.memset(epsc[:], EPS)

    # ---- pools (PSUM: mmp 2 + opp 2 + pop 2 + dpp 2 = 8 banks) ----
    mmp = ctx.enter_context(tc.tile_pool(name="mmp", bufs=2, space="PSUM"))
    opp = ctx.enter_context(tc.tile_pool(name="opp", bufs=2, space="PSUM"))
    pop = ctx.enter_context(tc.tile_pool(name="pop", bufs=2, space="PSUM"))
    dpp = ctx.enter_context(tc.tile_pool(name="dpp", bufs=2, space="PSUM"))

    xp = ctx.enter_context(tc.tile_pool(name="xp", bufs=2))
    sqp = ctx.enter_context(tc.tile_pool(name="sqp", bufs=2))
    stdp = ctx.enter_context(tc.tile_pool(name="stdp", bufs=2))
    rstdp = ctx.enter_context(tc.tile_pool(name="rstdp", bufs=2))
    rsbp = ctx.enter_context(tc.tile_pool(name="rsbp", bufs=2))
    tcp = ctx.enter_context(tc.tile_pool(name="tcp", bufs=2))
    bcbp = ctx.enter_context(tc.tile_pool(name="bcbp", bufs=2))
    esp = ctx.enter_context(tc.tile_pool(name="esp", bufs=3))
    recp = ctx.enter_context(tc.tile_pool(name="recp", bufs=2))
    rebp = ctx.enter_context(tc.tile_pool(name="rebp", bufs=2))
    bcsp = ctx.enter_context(tc.tile_pool(name="bcsp", bufs=2))
    stgp = ctx.enter_context(tc.tile_pool(name="stgp", bufs=2))
    ovp = ctx.enter_context(tc.tile_pool(name="ovp", bufs=2))

    # ================= projection + RMSNorm + RoPE, per 512-col chunk ======
    def proj_chunk(sc):
        cs = slice(sc * 512, (sc + 1) * 512)
        xc = xp.tile([P, 16, 512], bf16, tag="xc", name="xc")
        nc.sync.dma_start(xc[:], io["xt"][:, :, cs])

        def norm_rope(ps, m, nh, sumsel, bcsel, T):
            # ps: [m, 512] psum f32 proj result; T: SBUF bf16 dest [m, 512]
            sq = sqp.tile([P, 512], bf16, tag="sq", name="sq")[:m]
            nc.scalar.activation(sq, ps, Square)
            ssps = dpp.tile([P, 512], f32, tag="dp", name="ssps")[:nh]
            nc.tensor.matmul(ssps, sumsel, sq, start=True, stop=True)
            std = stdp.tile([2, 512], f32, tag="std", name="std")[:nh]
            nc.scalar.activation(std, ssps, Sqrt, bias=epsc[:nh], scale=1.0 / HD)
            rstd = rstdp.tile([2, 512], f32, tag="rstd", name="rstd")[:nh]
            nc.vector.reciprocal_approx_fast(rstd, std)
            rstdb = rsbp.tile([2, 512], bf16, tag="rstdb", name="rstdb")[:nh]
            nc.vector.tensor_copy(rstdb, rstd)
            bc = dpp.tile([P, 512], f32, tag="dp", name="bc")[:m]
            nc.tensor.matmul(bc, bcsel, rstdb, start=True, stop=True)
            # PSUM has a single DVE read port: stage bc in SBUF (ACT is the
            # engine closest to PSUM), then multiply against the psum ps.
            bcb = bcbp.tile([P, 512], bf16, tag="bcb", name="bcb")[:m]
            nc.scalar.copy(bcb, bc)
            # T = ps * bcast(g * rstd)   (g is folded into bcsel on the host)
            nc.vector.tensor_mul(T, ps, bcb)
            # RoPE in place on T
            tmpc = tcp.tile([P, 512], bf16, tag="tc", name="tmpc")[:m]
            nc.vector.tensor_mul(tmpc, T, cos4[:m, cs])
            sw = dpp.tile([P, 512], f32, tag="dp", name="sw")[:m]
            nc.tensor.matmul(sw, rot2[:m, :m], T, start=True, stop=True)
            nc.vector.tensor_mul(T, sw, sin4s[:m, cs])
            nc.vector.tensor_add(T, T, tmpc)

        def proj(w, m, pool):
            ps = pool.tile([P, 512], f32, tag=pool.name, name="ps")[:m]
            for kc in range(16):
                nc.tensor.matmul(
                    ps, w[:, kc, :], xc[:, kc, :],
                    start=(kc == 0), stop=(kc == 15),
                )
            return ps

        # K first (longest chain: norm+rope+dup), then Q pair 0/1, then V
        psk = proj(wk, HD, mmp)
        norm_rope(psk, HD, 1, onesk[:, :], selk[:, :], KT[0:HD, cs])
        nc.sync.dma_start(KT[HD:P, cs], KT[0:HD, cs])

        ps0 = proj(wqa, P, opp)
        norm_rope(ps0, P, 2, ones2[:, :], sel2g[:, :], QT[0][:, cs])
        ps1 = proj(wqb, P, mmp)
        norm_rope(ps1, P, 2, ones2[:, :], sel2g[:, :], QT[1][:, cs])

        for ms in range(4):
            pv = dpp.tile([P, 512], f32, tag="dp", name="pv")[:, 0:HD]
            for kc in range(16):
                nc.tensor.matmul(
                    pv, xc[:, kc, ms * P : (ms + 1) * P], wv[:, kc, :],
                    start=(kc == 0), stop=(kc == 15),
                )
            nc.scalar.copy(V[:, sc * 4 + ms, 0:HD], pv)

    # ================= attention + interleaved out-projection ==============
    def outproj_unit(qc, ms, dc):
        sl = slice(qc * 512 + ms * P, qc * 512 + (ms + 1) * P)
        pso = opp.tile([P, 512], f32, tag="opp", name="pso")
        for kc in range(2):
            nc.tensor.matmul(
                pso, OT[:, kc, sl], wo[:, kc, dc * 512 : (dc + 1) * 512],
                start=(kc == 0), stop=(kc == 1),
            )
        ov = ovp.tile([P, 512], bf16, tag="ov", name="ov")
        # alternate the psum->sbuf drain between DVE and ACT
        if (ms + dc) % 2:
            nc.scalar.copy(ov[:], pso[:])
        else:
            nc.vector.tensor_copy(ov[:], pso[:])
        nc.sync.dma_start(io["out"][sl, dc * 512 : (dc + 1) * 512], ov[:])

    def attn_chunk(qc, pending):
        # pending: list of (ms, dc) outproj units of chunk qc-1 to interleave
        qs = slice(qc * 512, (qc + 1) * 512)
        nkb = 4 * qc + 4
        stride = max(1, (nkb * 4) // 16)  # kb slots per interleaved unit
        slot = 0

        def tick():
            nonlocal slot
            slot += 1
            if pending and slot % stride == 0:
                outproj_unit(*pending.pop(0))

        for h in range(4):
            pair, poff = h // 2, (h % 2) * HD
            Q = QT[pair]
            po = pop.tile([HD + 1, 512], f32, tag="po", name="po")

            def score_exp(kb):
                # diagonal blocks (o >= 0): only columns >= 128*o can attend
                # to this key block -> narrow the score/exp/PV to [co:512]
                o = kb - 4 * qc
                co = max(0, o) * P
                ps = mmp.tile([P, 512], f32, tag="mmp", name="ps")
                nc.tensor.matmul(
                    ps[:, co:512],
                    KT[poff : poff + HD, kb * P : (kb + 1) * P],
                    Q[poff : poff + HD, qc * 512 + co : (qc + 1) * 512],
                    start=True, stop=True,
                )
                es = esp.tile([P, 512], bf16, tag="es", name="es")
                nc.scalar.activation(es[:, co:512], ps[:, co:512], Exp, scale=SCALE)
                if o >= 0:
                    # triangular mask on the 128-col diagonal sub-block
                    nc.vector.tensor_mul(es[:, co : co + P], es[:, co : co + P], tri)
                return es, co

            def pv(kb, es, co):
                nc.tensor.matmul(
                    po[:, co:512], V[:, kb, :], es[:, co:512],
                    start=(kb == 0), stop=(kb == nkb - 1),
                    skip_group_check=True,
                )

            prev, pco = score_exp(0)
            for kb in range(1, nkb):
                cur, cco = score_exp(kb)
                pv(kb - 1, prev, pco)
                tick()
                prev, pco = cur, cco
            pv(nkb - 1, prev, pco)
            tick()

            # normalize: row HD of po holds the softmax denominators
            den = recp.tile([1, 512], f32, tag="den", name="den")
            nc.scalar.copy(den, po[HD : HD + 1, :])
            rec = recp.tile([1, 512], f32, tag="rec", name="rec")
            nc.vector.reciprocal_approx_fast(rec, den)
            recb = rebp.tile([1, 512], bf16, tag="recb", name="recb")
            nc.vector.tensor_copy(recb, rec)
            bca = dpp.tile([P, 512], f32, tag="dp", name="bca")[:HD]
            nc.tensor.matmul(bca, sel1[:, :], recb, start=True, stop=True)
            bcs = bcsp.tile([HD, 512], bf16, tag="bcs", name="bcs")
            nc.scalar.copy(bcs, bca)
            if poff == 0:
                nc.vector.tensor_mul(OT[0:HD, pair, qs], po[0:HD, :], bcs)
            else:
                stg = stgp.tile([HD, 512], bf16, tag="stg", name="stg")
                nc.vector.tensor_mul(stg, po[0:HD, :], bcs)
                nc.sync.dma_start(OT[HD:P, pair, qs], stg[:])

        while pending:
            outproj_unit(*pending.pop(0))

    for sc in range(4):
        proj_chunk(sc)
    units = [(ms, dc) for ms in range(4) for dc in range(4)]
    for qc in range(4):
        attn_chunk(qc, [(qc - 1, ms, dc) for (ms, dc) in units] if qc else [])
    for ms, dc in units:
        outproj_unit(3, ms, dc)


def _prep_core_inputs(i, x, cos, sin, g_q, g_k, Wq, Wk, Wv, Wo):
    c0 = i * 4 * HD
    k0 = i * HD

    def b(a):
        return np.ascontiguousarray(a).astype(BF)

    x2 = x.reshape(S, D)
    xt = x2.reshape(S, 16, P).transpose(2, 1, 0)  # [p, kc, s]
    wqa = Wq[:, c0 : c0 + P].reshape(16, P, P).transpose(1, 0, 2)
    wqb = Wq[:, c0 + P : c0 + 2 * P].reshape(16, P, P).transpose(1, 0, 2)
    wk = Wk[:, k0 : k0 + HD].reshape(16, P, HD).transpose(1, 0, 2)
    wv = Wv[:, k0 : k0 + HD].reshape(16, P, HD).transpose(1, 0, 2)
    wo = Wo[c0 : c0 + 2 * P, :].reshape(2, P, D).transpose(1, 0, 2)
    cosT = cos.T.astype(np.float32)  # [32, S]
    sinT = sin.T.astype(np.float32)
    cos4 = np.tile(cosT, (4, 1))
    sin4s = np.concatenate([-sinT, sinT, -sinT, sinT], axis=0)
    tri = np.triu(np.ones((P, P), dtype=np.float32))  # [k within blk, q within blk]
    ones2 = np.zeros((P, 2), dtype=np.float32)
    ones2[:HD, 0] = 1.0
    ones2[HD:, 1] = 1.0
    sel2g = np.zeros((2, P), dtype=np.float32)
    sel2g[0, :HD] = g_q
    sel2g[1, HD:] = g_q
    r64 = np.roll(np.eye(HD, dtype=np.float32), 32, axis=0)
    rot2 = np.zeros((P, P), dtype=np.float32)
    rot2[:HD, :HD] = r64
    rot2[HD:, HD:] = r64
    return {
        "xt": b(xt),
        "wqa": b(wqa), "wqb": b(wqb), "wk": b(wk), "wv": b(wv), "wo": b(wo),
        "cos4": b(cos4), "sin4s": b(sin4s), "tri": b(tri),
        "ones2": b(ones2),
        "onesk": b(np.ones((HD, 1), dtype=np.float32)),
        "sel2g": b(sel2g),
        "selk": b(np.asarray(g_k, dtype=np.float32).reshape(1, HD)),
        "sel1": b(np.ones((1, HD), dtype=np.float32)),
        "rot2": b(rot2),
    }


def kernel(x, cos, sin, g_q, g_k, Wq, Wk, Wv, Wo):
    global LAST_RESULTS
    from concourse.bass_utils import run_bass_kernel_spmd

    if "nc" not in _CACHE:
        _CACHE["nc"] = _build_nc()
    nc = _CACHE["nc"]

    args = [np.asarray(a, dtype=np.float32) for a in
            (x, cos, sin, g_q, g_k, Wq, Wk, Wv, Wo)]
    in_maps = [_prep_core_inputs(i, *args) for i in range(N_CORES)]
    trace = bool(os.environ.get("BASS_TRACE"))
    res = run_bass_kernel_spmd(nc, in_maps, list(range(N_CORES)), trace=trace)
    LAST_RESULTS = res
    out = np.zeros((S, D), dtype=np.float32)
    for r in res.results:
        out += np.asarray(r["out"], dtype=np.float32)
    return out.reshape(1, S, D)
